# revision 24
# baseline (speedup 1.0000x reference)
"""Trainium2 Bass kernel for nn_Attention_47682726920277.

Causal multi-head attention with RoPE:
  q/k/v = x @ w{q,k,v}.T ; RoPE(q, k) ; att = softmax(mask(q k^T / 8)) ; out = (att v) @ wo.T
Shapes: x [2, 2048, 1024], 16 heads of dim 64, fp32.

Sharding (8 cores): data-parallel over batch (2) x tensor-parallel over heads (4 per
core). Each core computes its 4 heads' attention and a partial out via its wo row
block; the final all-reduce is the host-side sum of the 4 partials per batch.

Per-core pipeline (v3):
  - Host passes x^T and weight slices pre-transposed as float16.
  - Q,K are produced transposed (QT/KT [256ch, T]) so scores come out as S^T [k, q];
    softmax needs no max-subtraction (logits are small) and the denominator comes
    from a ones column appended to V (row 64 of the PV accumulator = sum_k exp).
  - Causal masking of diagonal 128x128 blocks is a gpsimd (Pool) 0/1-triangle
    multiply on the exp'd probabilities (pt), not PE mask matmuls: saves ~16us
    of tensor-engine time; diagonal k-blocks are processed FIRST within each
    unit so the exp->pool->PV chain hides behind subsequent full blocks.
  - Softmax normalization: Ln+Exp on the scalar engine (one ACT table set),
    gpsimd partition_broadcast, one DVE multiply.
  - Front: resident loads are split across three DMA queues (sync/scalar/pool)
    in strict need-order so the first projection's inputs (wq, x[:,0]) get the
    full HBM bandwidth instead of round-robin sharing with later loads.
  - Emission is a software pipeline over (head-pair, q-chunk) units with PE
    filler work injected inside each unit's key-block loop.
  - Output partials are stored as fp16; the host accumulates in fp32.
"""
import sys
import types
import numpy as np

B = 2
T = 2048
D = 1024
H = 16
HD = 64
NCORES = 8
GROUPS = NCORES // B          # head-groups per batch
HPC = H // GROUPS             # heads per core = 4
CH = HPC * HD                 # channels per core = 256
NQ = 512                      # PSUM bank width (fp32)
P = 128

_prog_cache = {}


def _install_ntff_shim():
    """The agent image's antenv lacks axon_hooks; inject it so trace=True works."""
    try:
        import antenv.axon_hooks  # noqa: F401
        return
    except ImportError:
        pass
    try:
        import trn_agent_boot.trn_boot as tb
        hook = tb._ntff_profile_via_ctypes('/opt/axon/libaxon_pjrt.so')
        if hook is None:
            return
        mod = types.ModuleType('antenv.axon_hooks')
        mod.get_axon_ntff_profile_hook = lambda: hook
        mod.set_axon_ntff_profile_hook = lambda h: None
        sys.modules['antenv.axon_hooks'] = mod
        import antenv
        antenv.axon_hooks = mod
    except Exception:
        pass


def _build_program(causal: bool):
    import concourse.bass as bass  # noqa: F401
    from concourse import bacc
    import concourse.tile as tile
    from concourse import mybir

    F32 = mybir.dt.float32
    F16 = mybir.dt.float16
    AF = mybir.ActivationFunctionType
    MUL = mybir.AluOpType.mult
    ADD = mybir.AluOpType.add

    NT = T // NQ          # proj/attention q-chunks (4)
    NKB = T // P          # k-blocks (16)
    DB = D // P           # d-blocks (8)
    CB = CH // P          # channel blocks = head-pair blocks (2)

    nc = bacc.Bacc("TRN2", target_bir_lowering=False, debug=False)

    # The act-table insertion pass picks, per activation, the first table set
    # containing its function: Exp -> exp_and_others, Ln -> natural_log.  A
    # kernel using both thrashes table loads (~1.5us each, on the scalar
    # critical path).  Restrict Exp/Ln to the one set that has both, so the
    # whole program needs a single ACT_TABLE_LOAD.  (Set positions must be
    # preserved -- ids are positional -- so mutate contents, not the list.)
    from concourse import hw_specs
    _tabs = hw_specs.get_activation_tables(nc.m.arch)
    for _name, _fns in _tabs.items():
        if _name != "natural_log_exp_and_others":
            _fns.discard(mybir.ActivationFunctionType.Exp)
            _fns.discard(mybir.ActivationFunctionType.Ln)

    # packed layouts: partition-major with contiguous per-partition runs so
    # each DMA descriptor covers KBs, not 1KB rows (front is BW-bound)
    xT = nc.dram_tensor("xT", [P, NT, DB, NQ], F16, kind="ExternalInput").ap()
    wqT = nc.dram_tensor("wqT", [P, DB, CH], F16, kind="ExternalInput").ap()
    wkT = nc.dram_tensor("wkT", [P, DB, CH], F16, kind="ExternalInput").ap()
    wvT = nc.dram_tensor("wvT", [P, DB, CH], F16, kind="ExternalInput").ap()
    woT = nc.dram_tensor("woT", [P, CB, D], F16, kind="ExternalInput").ap()
    cosT = nc.dram_tensor("cosT", [HD, T], F16, kind="ExternalInput").ap()
    sinS = nc.dram_tensor("sinS", [HD, T], F16, kind="ExternalInput").ap()
    ident = nc.dram_tensor("ident", [P, P], F16, kind="ExternalInput").ap()
    # -30000 strictly-below-diagonal mask, duplicated for both head planes so
    # one PE matmul masks a diagonal block's two halves
    triB2 = nc.dram_tensor("triB2", [P, 2, P], F16, kind="ExternalInput").ap()
    out = nc.dram_tensor("out", [T, D], F16, kind="ExternalOutput").ap()

    with tile.TileContext(nc) as tc:
        with tc.tile_pool(name="singles", bufs=1) as singles, \
             tc.tile_pool(name="rope16", bufs=4) as rope16, \
             tc.tile_pool(name="ptp", bufs=6) as ptp, \
             tc.tile_pool(name="obp", bufs=3) as obp, \
             tc.tile_pool(name="rcp", bufs=4) as rcp, \
             tc.tile_pool(name="bcp", bufs=4) as bcp, \
             tc.tile_pool(name="osp", bufs=4) as osp, \
             tc.tile_pool(name="st_ps", bufs=3, space="PSUM") as st_ps_pool, \
             tc.tile_pool(name="ot_ps", bufs=2, space="PSUM") as ot_ps_pool:

            # ---- resident tiles ----
            xT_sb = singles.tile([P, NT, DB, NQ], F16)
            wqT_sb = singles.tile([P, DB, CH], F16)
            wkT_sb = singles.tile([P, DB, CH], F16)
            wvT_sb = singles.tile([P, DB, CH], F16)
            woT_sb = singles.tile([P, CB, D], F16)
            cosT_sb = singles.tile([P, T], F16)
            sinS_sb = singles.tile([P, T], F16)
            ident_sb = singles.tile([P, P], F16)
            triB2_sb = singles.tile([P, 2, P], F16)

            # Front loads in strict need-order. Only SP (sync) and ACT
            # (scalar) have hardware DGE queues; within a queue descriptors
            # run in order, so each queue is a priority lane.  The first
            # projection's inputs (wq + x[:,0]) lead both lanes and the DMA
            # engine's round-robin gives them all the bandwidth; the first
            # proj matmuls (o=0..3) start on x0a alone via subtile deps.
            #   sync:   x0a, x0b          (RoPE swaps + out stores follow)
            #   scalar: wq, wk, cos, sin, trim, wv
            #   pool:   x1 (software DGE; x2/x3/wo follow as unit fill)
            nc.scalar.dma_start(wqT_sb[:], wqT[:])
            nc.sync.dma_start(xT_sb[:, 0, 0:4], xT[:, 0, 0:4])
            nc.sync.dma_start(xT_sb[:, 0, 4:8], xT[:, 0, 4:8])
            # warm exp: fires the ACT table load early.  Its source is a
            # memset tile, NOT a DMA'd tile -- the scalar queue is in-order,
            # so an exp waiting on a DMA would head-of-line block every
            # scalar-queue load behind it (this cost the old kernel ~4us:
            # wk/wv sat behind a warm exp that waited for the cos transfer).
            warmsrc = rcp.tile([1, NQ], F32, tag="lnr", name="warmsrc")
            nc.vector.memset(warmsrc[:, 0:8], 0.0)
            warm = rcp.tile([1, NQ], F32, tag="lnr", name="warm")
            nc.scalar.activation(warm[:, 0:8], warmsrc[:, 0:8], AF.Exp)
            # The DMA hardware round-robins bandwidth across ALL issued
            # descriptors regardless of queue, so issuing later loads early
            # starves the front-critical wq+x0 transfers.  Pool-queue DMAs
            # can't be throttled by compute deps at all (software DGE issues
            # regardless), so every front-ordered load rides the scalar HW
            # queue, gated behind a stall chain: one op waiting on the x0
            # transfers plus three chained dummies to defeat the engines'
            # 4-deep relaxed-ordering lookahead.
            stall_s = rope16.tile([1, 2, 8], F16, tag="stall", name="stall_s")
            # read spans db=3 (x0a) AND db=4 (x0b) so the dependency covers
            # both x0 DMAs no matter how subtile attribution resolves
            nc.scalar.copy(stall_s[:], xT_sb[0:1, 0, 3:5, 0:8])
            for _i in range(3):
                nc.scalar.copy(stall_s[:], stall_s[:])
            nc.scalar.dma_start(wkT_sb[:], wkT[:])
            nc.scalar.dma_start(cosT_sb[0:HD, :], cosT[:])
            nc.scalar.dma_start(sinS_sb[0:HD, :], sinS[:])
            nc.scalar.dma_start(xT_sb[:, 1, 0:4], xT[:, 1, 0:4])
            nc.scalar.dma_start(xT_sb[:, 1, 4:8], xT[:, 1, 4:8])
            nc.scalar.dma_start(ident_sb[:], ident[:])
            nc.scalar.dma_start(triB2_sb[:], triB2[:])
            nc.scalar.dma_start(wvT_sb[:], wvT[:])
            nc.scalar.dma_start(xT_sb[:, 2], xT[:, 2])
            # cos/sin rows 64-127 duplicate rows 0-63 (c % 64 layout)
            nc.vector.tensor_copy(cosT_sb[HD:P, :], cosT_sb[0:HD, :])
            nc.vector.tensor_copy(sinS_sb[HD:P, :], sinS_sb[0:HD, :])
            # row-swapped sin copy for the low-latency RoPE variant
            sinW_sb = singles.tile([P, T], F16)
            for g in range(4):
                nc.vector.tensor_copy(sinW_sb[(g ^ 1) * 32:(g ^ 1) * 32 + 32, :],
                                      sinS_sb[g * 32:g * 32 + 32, :])

            QT_sb = singles.tile([P, CB, T], F16)
            KT_sb = singles.tile([P, CB, T], F16)
            attnT_sb = singles.tile([P, CB, T], F16)
            # V with a ones column per head: [kb, head, 65].  memset, not DMA:
            # a strided 2-byte-element DMA is a descriptor bomb that occupies
            # its issue queue for ~11us.
            vaug = singles.tile([P, NKB, HPC, HD + 1], F16)
            nc.vector.memset(vaug[:, :, :, HD:HD + 1], 1.0)

            # ---- projections (one head-pair block, one q-chunk), RoPE fused ----
            def proj_chunk(w_sb, dst_sb, cb, m, pname, dma_swap=True):
                ps = st_ps_pool.tile([P, 2, NQ], F32, tag="st",
                                     name=f"prj_{pname}_{cb}_{m}")[:, 0, :]
                for o in range(DB):
                    nc.tensor.matmul(
                        ps[:],
                        w_sb[:, o, cb * P:(cb + 1) * P],
                        xT_sb[:, m, o, :],
                        start=(o == 0), stop=(o == DB - 1))
                cs = slice(m * NQ, (m + 1) * NQ)
                # q' = q*cos + swap32(q)*sinS (sign folded into the sin table).
                qraw = rope16.tile([P, NQ], F16, tag="qraw",
                                   name=f"qr_{pname}_{cb}_{m}")
                nc.vector.tensor_copy(qraw[:], ps[:])
                nc.vector.tensor_tensor(dst_sb[:, cb, cs], qraw[:],
                                        cosT_sb[:, cs], MUL)
                tmp = rope16.tile([P, NQ], F16, tag="tmp",
                                  name=f"tm_{pname}_{cb}_{m}")
                if dma_swap:
                    # partition swap on the DMA engines (sync queue, hardware
                    # DGE) -> the DVE does one 2x-rate fp16 multiply
                    qsw = rope16.tile([P, NQ], F16, tag="qsw",
                                      name=f"qs_{pname}_{cb}_{m}")
                    for g in range(4):
                        src = (g ^ 1) * 32
                        dst = g * 32
                        nc.sync.dma_start(qsw[dst:dst + 32, :],
                                          qraw[src:src + 32, :])
                    nc.vector.tensor_tensor(tmp[:], qsw[:],
                                            sinS_sb[:, cs], MUL)
                else:
                    # low-latency variant for the pipeline-fill chunks: four
                    # cross-partition DVE multiplies, no DMA hop.  sinW is
                    # row-swapped so both inputs read the same partitions.
                    for g in range(4):
                        src = (g ^ 1) * 32
                        dst = g * 32
                        nc.vector.tensor_tensor(
                            tmp[dst:dst + 32, :],
                            qraw[src:src + 32, :],
                            sinW_sb[src:src + 32, cs], MUL)
                nc.vector.tensor_tensor(dst_sb[:, cb, cs],
                                        dst_sb[:, cb, cs], tmp[:], ADD)

            def proj_v(iis):
                for i in iis:
                    ps = st_ps_pool.tile([P, 2, NQ], F32, tag="st",
                                         name=f"v_{i}")[:, 0, :]
                    vps = ps[:, :CH]
                    for o in range(DB):
                        nc.tensor.matmul(
                            vps,
                            xT_sb[:, i // 4, o, (i % 4) * P:(i % 4 + 1) * P],
                            wvT_sb[:, o, :],
                            start=(o == 0), stop=(o == DB - 1))
                    # alternate the PSUM->SBUF drain between scalar and DVE
                    if i % 2 == 0:
                        nc.scalar.copy(
                            vaug[:, i, :, 0:HD],
                            vps.rearrange("p (h d) -> p h d", h=HPC))
                    else:
                        nc.vector.tensor_copy(
                            vaug[:, i, :, 0:HD],
                            vps.rearrange("p (h d) -> p h d", h=HPC))

            def wo_unit(i, j, copy_eng="v"):
                ps = st_ps_pool.tile([P, 2, NQ], F32, tag="st",
                                     name=f"o_{i}_{j}")[:, 0, :]
                for cb in range(CB):
                    nc.tensor.matmul(
                        ps[:],
                        attnT_sb[:, cb, i * P:(i + 1) * P],
                        woT_sb[:, cb, j * NQ:(j + 1) * NQ],
                        start=(cb == 0), stop=(cb == CB - 1))
                ob = obp.tile([P, NQ], F16, tag="ob", name=f"ob_{i}_{j}")
                if copy_eng == "s":
                    nc.scalar.copy(ob[:], ps[:])
                else:
                    nc.vector.tensor_copy(ob[:], ps[:])
                nc.sync.dma_start(
                    out[i * P:(i + 1) * P, j * NQ:(j + 1) * NQ], ob[:])

            def kb_list(qc):
                # diagonal blocks LAST: the pool queue is in-order, and a
                # unit's mask ops must come before its normalize broadcasts --
                # diag-first would make unit N+1's first PV wait (via its pool
                # mask, queued behind unit N's broadcast) for unit N's entire
                # normalize chain, serializing the units.
                return list(range(min(NKB, (qc + 1) * (NQ // P)))) if causal \
                    else list(range(NKB))

            # ---- one attention unit: scores+exp+PV+normalize for (hp, qc) ----
            # fill: list of closures emitting PE filler work, spread across the
            # kb loop so the tensor engine stays busy while exp runs.
            def unit(hp, qc, fill=()):
                kbs = kb_list(qc)
                q0 = qc * NQ
                fill = list(fill)
                # pop filler roughly evenly across the kb loop
                fill_at = {}
                for fi in range(len(fill)):
                    pos = min(len(kbs) - 1, (fi * len(kbs)) // max(1, len(fill)))
                    fill_at.setdefault(pos, []).append(fill[fi])

                otps = [ot_ps_pool.tile([HD + 1, NQ], F32, tag="ot",
                                        name=f"ot_{hp}_{qc}_{i}")
                        for i in range(2)]

                def finish(kb, stp2, qsl, diag, first, last):
                    pt = ptp.tile([P, 2, NQ], F16, tag="pt",
                                  name=f"pt_{hp}_{qc}_{kb}")
                    sflat = stp2.rearrange("p a b -> p (a b)")
                    pflat = pt.rearrange("p a b -> p (a b)")
                    # one exp covers both halves; the uncomputed middle
                    # columns of diagonal blocks are never read downstream
                    nc.scalar.activation(pflat[:, qsl:2 * NQ],
                                         sflat[:, qsl:2 * NQ],
                                         AF.Exp, scale=float(HD) ** -0.5)
                    for half in range(2):
                        h = hp * 2 + half
                        nc.tensor.matmul(
                            otps[half][:, qsl:NQ],
                            vaug[:, kb, h, :],
                            pt[:, half, qsl:NQ],
                            start=first, stop=last)

                pend = None
                for ki, kb in enumerate(kbs):
                    qsl = max(0, kb * P - q0) if causal else 0
                    diag = causal and kb * P >= q0
                    stp2 = st_ps_pool.tile([P, 2, NQ], F32, tag="st",
                                           name=f"st_{hp}_{qc}_{kb}")
                    for half in range(2):
                        hb = half * HD
                        nc.tensor.matmul(
                            stp2[:, half, qsl:NQ],
                            KT_sb[hb:hb + HD, hp, kb * P:(kb + 1) * P],
                            QT_sb[hb:hb + HD, hp, q0 + qsl:q0 + NQ],
                            start=True, stop=not diag)
                    if diag:
                        # causal mask: add -30000 strictly below the diagonal
                        # (both head planes in ONE matmul) so exp underflows
                        # those elements to zero
                        nc.tensor.matmul(
                            stp2[:, :, qsl:qsl + P],
                            ident_sb[:],
                            triB2_sb[:],
                            start=False, stop=True)
                    for f in fill_at.get(ki, ()):
                        f()
                    if pend is not None:
                        finish(*pend)
                    pend = (kb, stp2, qsl, diag, ki == 0, ki == len(kbs) - 1)
                finish(*pend)

                # normalize: one copy drains the PV accumulator to SBUF
                # (frees the PSUM bank early for the next unit's PV); then
                # 1/sums = exp(-ln(sums)) on the scalar engine (single table
                # set, see above), gpsimd partition-broadcast, one DVE multiply.
                last = (hp == 1 and qc == NT - 1)
                for half in range(2):
                    if not last:
                        osb = osp.tile([HD + 1, NQ], F16, tag="osb",
                                       name=f"os_{hp}_{qc}_{half}")
                        nc.vector.tensor_copy(osb[:], otps[half][:, :])
                        src_s, src_o = osb[HD:HD + 1, :], osb[0:HD, :]
                    else:
                        # final unit: skip the SBUF staging hop -- nothing
                        # needs the PSUM bank after this, and the tail is
                        # latency-critical
                        src_s, src_o = otps[half][HD:HD + 1, :], \
                            otps[half][0:HD, :]
                    lnr = rcp.tile([1, NQ], F32, tag="lnr",
                                   name=f"ln_{hp}_{qc}_{half}")
                    nc.scalar.activation(lnr[:], src_s, AF.Ln)
                    rcd = rcp.tile([1, NQ], F16, tag="rcd",
                                   name=f"rc_{hp}_{qc}_{half}")
                    nc.scalar.activation(rcd[:], lnr[:], AF.Exp, scale=-1.0)
                    bc = bcp.tile([HD, NQ], F16, tag="bc",
                                  name=f"bc_{hp}_{qc}_{half}")
                    nc.gpsimd.partition_broadcast(bc[:], rcd[:])
                    nc.vector.tensor_tensor(
                        attnT_sb[half * HD:(half + 1) * HD, hp,
                                 q0:q0 + NQ],
                        src_o, bc[:], MUL)

            # ---- emission: streaming pipeline ----
            # stage deps: unit(0,m) needs Q0/K0 chunks <= m and V blocks < 4(m+1);
            # unit(1,m) needs Q1/K1 chunks <= m; wo(qc) needs both hps normalized.
            def pq(cb, m, dma_swap=True):
                return lambda: proj_chunk(wqT_sb, QT_sb, cb, m, "q", dma_swap)

            def pk(cb, m, dma_swap=True):
                return lambda: proj_chunk(wkT_sb, KT_sb, cb, m, "k", dma_swap)

            def pv(iis):
                return lambda: proj_v(iis)

            def wo(i, j):
                return lambda: wo_unit(i, j)

            def xload(m):
                return lambda: nc.sync.dma_start(xT_sb[:, m], xT[:, m])

            def woload():
                return lambda: nc.sync.dma_start(woT_sb[:], woT[:])

            # projections are emitted one unit ahead of the unit that consumes
            # them, so their RoPE chain (DVE + swap DMAs) completes off the
            # critical path.  x3/wo load as sync-queue fill: pool-queue DMA
            # issues can't be ordered (software DGE fires immediately and
            # floods the front), and the scalar queue must stay clear for exp.
            pq(0, 0, False)(); pk(0, 0, False)()
            unit(0, 0, [pv([0, 1]), pv([2, 3]), pq(0, 1), pk(0, 1),
                        pv([4, 5]), pv([6, 7]), pq(0, 2)])
            unit(0, 1, [pv([8, 9]), xload(3), pv([10, 11]), pk(0, 2),
                        pq(0, 3)])
            unit(0, 2, [pv([12, 13]), pv([14, 15]), pk(0, 3), pq(1, 0)])
            unit(0, 3, [woload(), pk(1, 0), pq(1, 1), pk(1, 1)])
            unit(1, 0, [pq(1, 2), pk(1, 2)])
            # wo fills lag their normalize by ~a unit so they never head-of-
            # line block the PE queue behind an in-flight normalize chain
            unit(1, 1, [pq(1, 3), pk(1, 3), wo(0, 0)])
            unit(1, 2, [wo(0, 1), wo(1, 0), wo(1, 1), wo(2, 0), wo(2, 1),
                        wo(3, 0), wo(3, 1)])
            unit(1, 3, [wo(4, 0), wo(4, 1), wo(5, 0), wo(5, 1),
                        wo(6, 0), wo(6, 1), wo(7, 0), wo(7, 1),
                        wo(8, 0), wo(8, 1), wo(9, 0), wo(9, 1)])
            # i=10,11 only need norm(*,2): they give the PE work to chew
            # while the final unit's normalize chain runs
            for i in (10, 11):
                wo_unit(i, 0)
                wo_unit(i, 1)
            # tail: scalar engine is free after the last exp -- put the
            # drains there so the DVE isn't the tail pacer
            for i in range(12, 16):
                wo_unit(i, 0, copy_eng="s")
                wo_unit(i, 1, copy_eng="v")

    nc.compile()
    return nc


def _get_program(causal: bool):
    key = ("causal" if causal else "full")
    if key not in _prog_cache:
        _prog_cache[key] = _build_program(causal)
    return _prog_cache[key]


def _mask_kind(mask):
    m = np.asarray(mask)
    if m.ndim == 4:
        m = m[0, 0]
    if (m != 0).all():
        return False  # full attention
    trilm = np.tril(np.ones((m.shape[0], m.shape[1]), dtype=m.dtype))
    if np.array_equal(m, trilm):
        return True
    raise NotImplementedError("mask is neither all-ones nor causal tril")


def _make_in_maps(x, cos, sin, wq, wk, wv, wo):
    x = np.asarray(x, dtype=np.float32)
    cos = np.asarray(cos, dtype=np.float32)
    sin = np.asarray(sin, dtype=np.float32)
    wq = np.asarray(wq, dtype=np.float32)
    wk = np.asarray(wk, dtype=np.float32)
    wv = np.asarray(wv, dtype=np.float32)
    wo = np.asarray(wo, dtype=np.float32)

    # RoPE tables in transposed head-pair layout [128ch, T].
    # cos2T[c, t] = cos[t, c % 64]; sinS flips sign on the low half of each
    # head (the kernel swaps q's partner rows with a DMA, so the sin table is
    # in natural row order).
    cos2T = np.ascontiguousarray(cos[:T, :].T.astype(np.float16))   # [64, T]
    sgn = np.where(np.arange(HD) < (HD // 2), -1.0, 1.0).astype(np.float32)
    sinS = np.ascontiguousarray(
        (sin[:T, :].T * sgn[:, None]).astype(np.float16))      # [64, T]
    identm = np.eye(P, dtype=np.float16)
    triB = (np.tril(np.ones((P, P), np.float32), -1) * -30000.0).astype(np.float16)
    triB2 = np.ascontiguousarray(
        np.broadcast_to(triB[:, None, :], (P, 2, P)).copy())

    def pack_w(wT):
        # [D, CH] -> [128, DB, CH] with row p = wT[o*128+p, :]
        return np.ascontiguousarray(
            wT.reshape(D // P, P, -1).transpose(1, 0, 2).astype(np.float16))

    in_maps = []
    for core in range(NCORES):
        b = core // GROUPS
        g = core % GROUPS
        c0 = g * CH
        xb = x[b].T.astype(np.float16)                 # [D, T]
        xpack = np.ascontiguousarray(
            xb.reshape(D // P, P, T // NQ, NQ).transpose(1, 2, 0, 3))
        in_maps.append({
            "xT": xpack,                                        # [128, NT, DB, NQ]
            "wqT": pack_w(wq[c0:c0 + CH, :].T),
            "wkT": pack_w(wk[c0:c0 + CH, :].T),
            "wvT": pack_w(wv[c0:c0 + CH, :].T),
            "woT": np.ascontiguousarray(                       # [128, CB, D]
                wo[:, c0:c0 + CH].T.reshape(CH // P, P, D)
                .transpose(1, 0, 2).astype(np.float16)),
            "cosT": cos2T,
            "sinS": sinS,
            "ident": identm,
            "triB2": triB2,
        })
    return in_maps


def _run(inputs, trace=False):
    from concourse import bass_utils
    causal = _mask_kind(inputs["mask"])
    nc = _get_program(causal)
    in_maps = _make_in_maps(
        inputs["x"], inputs["cos"], inputs["sin"],
        inputs["wq"], inputs["wk"], inputs["wv"], inputs["wo"])
    if trace:
        _install_ntff_shim()
    res = bass_utils.run_bass_kernel_spmd(
        nc, in_maps, core_ids=list(range(NCORES)), trace=trace)
    outs = [r["out"] for r in res.results]
    full = np.empty((B, T, D), dtype=np.float32)
    for b in range(B):
        full[b] = outs[b * GROUPS].astype(np.float32)
        for g in range(1, GROUPS):
            full[b] += outs[b * GROUPS + g].astype(np.float32)
    return full, res


def kernel(**inputs):
    full, _ = _run(inputs, trace=False)
    return full


def kernel_profiled(**inputs):
    """Like kernel() but with NTFF tracing; returns (out, BassKernelResults)."""
    return _run(inputs, trace=True)


# revision 25
# speedup vs baseline: 1.0440x; 1.0440x over previous
"""Trainium2 Bass kernel for nn_Attention_47682726920277.

Causal multi-head attention with RoPE:
  q/k/v = x @ w{q,k,v}.T ; RoPE(q, k) ; att = softmax(mask(q k^T / 8)) ; out = (att v) @ wo.T
Shapes: x [2, 2048, 1024], 16 heads of dim 64, fp32.

Sharding (8 cores): data-parallel over batch (2) x tensor-parallel over heads (4 per
core). Each core computes its 4 heads' attention and a partial out via its wo row
block; the final all-reduce is the host-side sum of the 4 partials per batch.

Per-core pipeline (v3):
  - Host passes x^T and weight slices pre-transposed as float16.
  - Q,K are produced transposed (QT/KT [256ch, T]) so scores come out as S^T [k, q];
    softmax needs no max-subtraction (logits are small) and the denominator comes
    from a ones column appended to V (row 64 of the PV accumulator = sum_k exp).
  - Causal masking of diagonal 128x128 blocks is a gpsimd (Pool) 0/1-triangle
    multiply on the exp'd probabilities (pt), not PE mask matmuls: saves ~16us
    of tensor-engine time; diagonal k-blocks are processed FIRST within each
    unit so the exp->pool->PV chain hides behind subsequent full blocks.
  - Softmax normalization: Ln+Exp on the scalar engine (one ACT table set),
    gpsimd partition_broadcast, one DVE multiply.
  - Front: resident loads are split across three DMA queues (sync/scalar/pool)
    in strict need-order so the first projection's inputs (wq, x[:,0]) get the
    full HBM bandwidth instead of round-robin sharing with later loads.
  - Emission is a software pipeline over (head-pair, q-chunk) units with PE
    filler work injected inside each unit's key-block loop.
  - Output partials are stored as fp16; the host accumulates in fp32.
"""
import sys
import types
import numpy as np

B = 2
T = 2048
D = 1024
H = 16
HD = 64
NCORES = 8
GROUPS = NCORES // B          # head-groups per batch
HPC = H // GROUPS             # heads per core = 4
CH = HPC * HD                 # channels per core = 256
NQ = 512                      # PSUM bank width (fp32)
P = 128

_prog_cache = {}


def _install_ntff_shim():
    """The agent image's antenv lacks axon_hooks; inject it so trace=True works."""
    try:
        import antenv.axon_hooks  # noqa: F401
        return
    except ImportError:
        pass
    try:
        import trn_agent_boot.trn_boot as tb
        hook = tb._ntff_profile_via_ctypes('/opt/axon/libaxon_pjrt.so')
        if hook is None:
            return
        mod = types.ModuleType('antenv.axon_hooks')
        mod.get_axon_ntff_profile_hook = lambda: hook
        mod.set_axon_ntff_profile_hook = lambda h: None
        sys.modules['antenv.axon_hooks'] = mod
        import antenv
        antenv.axon_hooks = mod
    except Exception:
        pass


def _build_program(causal: bool):
    import concourse.bass as bass  # noqa: F401
    from concourse import bacc
    import concourse.tile as tile
    from concourse import mybir

    F32 = mybir.dt.float32
    F16 = mybir.dt.float16
    AF = mybir.ActivationFunctionType
    MUL = mybir.AluOpType.mult
    ADD = mybir.AluOpType.add

    NT = T // NQ          # proj/attention q-chunks (4)
    NKB = T // P          # k-blocks (16)
    DB = D // P           # d-blocks (8)
    CB = CH // P          # channel blocks = head-pair blocks (2)

    nc = bacc.Bacc("TRN2", target_bir_lowering=False, debug=False)

    # The act-table insertion pass picks, per activation, the first table set
    # containing its function: Exp -> exp_and_others, Ln -> natural_log.  A
    # kernel using both thrashes table loads (~1.5us each, on the scalar
    # critical path).  Restrict Exp/Ln to the one set that has both, so the
    # whole program needs a single ACT_TABLE_LOAD.  (Set positions must be
    # preserved -- ids are positional -- so mutate contents, not the list.)
    from concourse import hw_specs
    _tabs = hw_specs.get_activation_tables(nc.m.arch)
    for _name, _fns in _tabs.items():
        if _name != "natural_log_exp_and_others":
            _fns.discard(mybir.ActivationFunctionType.Exp)
            _fns.discard(mybir.ActivationFunctionType.Ln)

    # packed layouts: partition-major with contiguous per-partition runs so
    # each DMA descriptor covers KBs, not 1KB rows (front is BW-bound)
    xT = nc.dram_tensor("xT", [P, NT, DB, NQ], F16, kind="ExternalInput").ap()
    wqT = nc.dram_tensor("wqT", [P, DB, CH], F16, kind="ExternalInput").ap()
    wkT = nc.dram_tensor("wkT", [P, DB, CH], F16, kind="ExternalInput").ap()
    wvT = nc.dram_tensor("wvT", [P, DB, CH], F16, kind="ExternalInput").ap()
    woT = nc.dram_tensor("woT", [P, CB, D], F16, kind="ExternalInput").ap()
    cosT = nc.dram_tensor("cosT", [HD, T], F16, kind="ExternalInput").ap()
    sinS = nc.dram_tensor("sinS", [HD, T], F16, kind="ExternalInput").ap()
    ident = nc.dram_tensor("ident", [P, P], F16, kind="ExternalInput").ap()
    # -30000 strictly-below-diagonal mask, duplicated for both head planes so
    # one PE matmul masks a diagonal block's two halves
    triB2 = nc.dram_tensor("triB2", [P, 2, P], F16, kind="ExternalInput").ap()
    out = nc.dram_tensor("out", [T, D], F16, kind="ExternalOutput").ap()

    with tile.TileContext(nc) as tc:
        with tc.tile_pool(name="singles", bufs=1) as singles, \
             tc.tile_pool(name="rope16", bufs=4) as rope16, \
             tc.tile_pool(name="ptp", bufs=6) as ptp, \
             tc.tile_pool(name="obp", bufs=3) as obp, \
             tc.tile_pool(name="rcp", bufs=4) as rcp, \
             tc.tile_pool(name="bcp", bufs=4) as bcp, \
             tc.tile_pool(name="osp", bufs=4) as osp, \
             tc.tile_pool(name="st_ps", bufs=3, space="PSUM") as st_ps_pool, \
             tc.tile_pool(name="ot_ps", bufs=2, space="PSUM") as ot_ps_pool:

            # ---- resident tiles ----
            xT_sb = singles.tile([P, NT, DB, NQ], F16)
            wqT_sb = singles.tile([P, DB, CH], F16)
            wkT_sb = singles.tile([P, DB, CH], F16)
            wvT_sb = singles.tile([P, DB, CH], F16)
            woT_sb = singles.tile([P, CB, D], F16)
            cosT_sb = singles.tile([P, T], F16)
            sinS_sb = singles.tile([P, T], F16)
            ident_sb = singles.tile([P, P], F16)
            triB2_sb = singles.tile([P, 2, P], F16)

            # Front loads in strict need-order. Only SP (sync) and ACT
            # (scalar) have hardware DGE queues; within a queue descriptors
            # run in order, so each queue is a priority lane.  The first
            # projection's inputs (wq + x[:,0]) lead both lanes and the DMA
            # engine's round-robin gives them all the bandwidth; the first
            # proj matmuls (o=0..3) start on x0a alone via subtile deps.
            #   sync:   x0a, x0b          (RoPE swaps + out stores follow)
            #   scalar: wq, wk, cos, sin, trim, wv
            #   pool:   x1 (software DGE; x2/x3/wo follow as unit fill)
            nc.scalar.dma_start(wqT_sb[:], wqT[:])
            nc.sync.dma_start(xT_sb[:, 0, 0:4], xT[:, 0, 0:4])
            nc.sync.dma_start(xT_sb[:, 0, 4:8], xT[:, 0, 4:8])
            # warm exp: fires the ACT table load early.  Its source is a
            # memset tile, NOT a DMA'd tile -- the scalar queue is in-order,
            # so an exp waiting on a DMA would head-of-line block every
            # scalar-queue load behind it (this cost the old kernel ~4us:
            # wk/wv sat behind a warm exp that waited for the cos transfer).
            warmsrc = rcp.tile([1, NQ], F32, tag="lnr", name="warmsrc")
            nc.vector.memset(warmsrc[:, 0:8], 0.0)
            warm = rcp.tile([1, NQ], F32, tag="lnr", name="warm")
            nc.scalar.activation(warm[:, 0:8], warmsrc[:, 0:8], AF.Exp)
            # The DMA hardware round-robins bandwidth across ALL issued
            # descriptors regardless of queue, so issuing later loads early
            # starves the front-critical wq+x0 transfers.  Pool-queue DMAs
            # can't be throttled by compute deps at all (software DGE issues
            # regardless), so every front-ordered load rides the scalar HW
            # queue, gated behind a stall chain: one op waiting on the x0
            # transfers plus three chained dummies to defeat the engines'
            # 4-deep relaxed-ordering lookahead.
            stall_s = rope16.tile([1, 2, 8], F16, tag="stall", name="stall_s")
            # read spans db=3 (x0a) AND db=4 (x0b) so the dependency covers
            # both x0 DMAs no matter how subtile attribution resolves
            nc.scalar.copy(stall_s[:], xT_sb[0:1, 0, 3:5, 0:8])
            for _i in range(3):
                nc.scalar.copy(stall_s[:], stall_s[:])
            nc.scalar.dma_start(wkT_sb[:], wkT[:])
            nc.scalar.dma_start(cosT_sb[0:HD, :], cosT[:])
            nc.scalar.dma_start(sinS_sb[0:HD, :], sinS[:])
            nc.scalar.dma_start(xT_sb[:, 1, 0:4], xT[:, 1, 0:4])
            nc.scalar.dma_start(xT_sb[:, 1, 4:8], xT[:, 1, 4:8])
            nc.scalar.dma_start(ident_sb[:], ident[:])
            nc.scalar.dma_start(triB2_sb[:], triB2[:])
            nc.scalar.dma_start(wvT_sb[:], wvT[:])
            nc.scalar.dma_start(xT_sb[:, 2], xT[:, 2])
            # cos/sin rows 64-127 duplicate rows 0-63 (c % 64 layout)
            nc.vector.tensor_copy(cosT_sb[HD:P, :], cosT_sb[0:HD, :])
            nc.vector.tensor_copy(sinS_sb[HD:P, :], sinS_sb[0:HD, :])
            # row-swapped sin copy for the low-latency RoPE variant
            sinW_sb = singles.tile([P, T], F16)
            for g in range(4):
                nc.vector.tensor_copy(sinW_sb[(g ^ 1) * 32:(g ^ 1) * 32 + 32, :],
                                      sinS_sb[g * 32:g * 32 + 32, :])

            QT_sb = singles.tile([P, CB, T], F16)
            KT_sb = singles.tile([P, CB, T], F16)
            attnT_sb = singles.tile([P, CB, T], F16)
            # V with a ones column per head: [kb, head, 65].  memset, not DMA:
            # a strided 2-byte-element DMA is a descriptor bomb that occupies
            # its issue queue for ~11us.
            vaug = singles.tile([P, NKB, HPC, HD + 1], F16)
            nc.vector.memset(vaug[:, :, :, HD:HD + 1], 1.0)

            # ---- projections (one head-pair block, one q-chunk), RoPE fused ----
            def proj_chunk(w_sb, dst_sb, cb, m, pname, dma_swap=True):
                ps = st_ps_pool.tile([P, 2, NQ], F32, tag="st",
                                     name=f"prj_{pname}_{cb}_{m}")[:, 0, :]
                for o in range(DB):
                    nc.tensor.matmul(
                        ps[:],
                        w_sb[:, o, cb * P:(cb + 1) * P],
                        xT_sb[:, m, o, :],
                        start=(o == 0), stop=(o == DB - 1))
                cs = slice(m * NQ, (m + 1) * NQ)
                # q' = q*cos + swap32(q)*sinS (sign folded into the sin table).
                qraw = rope16.tile([P, NQ], F16, tag="qraw",
                                   name=f"qr_{pname}_{cb}_{m}")
                nc.vector.tensor_copy(qraw[:], ps[:])
                nc.vector.tensor_tensor(dst_sb[:, cb, cs], qraw[:],
                                        cosT_sb[:, cs], MUL)
                tmp = rope16.tile([P, NQ], F16, tag="tmp",
                                  name=f"tm_{pname}_{cb}_{m}")
                if dma_swap:
                    # partition swap on the DMA engines (sync queue, hardware
                    # DGE) -> the DVE does one 2x-rate fp16 multiply
                    qsw = rope16.tile([P, NQ], F16, tag="qsw",
                                      name=f"qs_{pname}_{cb}_{m}")
                    for g in range(4):
                        src = (g ^ 1) * 32
                        dst = g * 32
                        nc.sync.dma_start(qsw[dst:dst + 32, :],
                                          qraw[src:src + 32, :])
                    nc.vector.tensor_tensor(tmp[:], qsw[:],
                                            sinS_sb[:, cs], MUL)
                else:
                    # low-latency variant for the pipeline-fill chunks: four
                    # cross-partition DVE multiplies, no DMA hop.  sinW is
                    # row-swapped so both inputs read the same partitions.
                    for g in range(4):
                        src = (g ^ 1) * 32
                        dst = g * 32
                        nc.vector.tensor_tensor(
                            tmp[dst:dst + 32, :],
                            qraw[src:src + 32, :],
                            sinW_sb[src:src + 32, cs], MUL)
                nc.vector.tensor_tensor(dst_sb[:, cb, cs],
                                        dst_sb[:, cb, cs], tmp[:], ADD)

            def proj_v(iis):
                for i in iis:
                    ps = st_ps_pool.tile([P, 2, NQ], F32, tag="st",
                                         name=f"v_{i}")[:, 0, :]
                    vps = ps[:, :CH]
                    for o in range(DB):
                        nc.tensor.matmul(
                            vps,
                            xT_sb[:, i // 4, o, (i % 4) * P:(i % 4 + 1) * P],
                            wvT_sb[:, o, :],
                            start=(o == 0), stop=(o == DB - 1))
                    # alternate the PSUM->SBUF drain between scalar and DVE
                    if i % 2 == 0:
                        nc.scalar.copy(
                            vaug[:, i, :, 0:HD],
                            vps.rearrange("p (h d) -> p h d", h=HPC))
                    else:
                        nc.vector.tensor_copy(
                            vaug[:, i, :, 0:HD],
                            vps.rearrange("p (h d) -> p h d", h=HPC))

            def wo_unit(i, j, copy_eng="v"):
                ps = st_ps_pool.tile([P, 2, NQ], F32, tag="st",
                                     name=f"o_{i}_{j}")[:, 0, :]
                for cb in range(CB):
                    nc.tensor.matmul(
                        ps[:],
                        attnT_sb[:, cb, i * P:(i + 1) * P],
                        woT_sb[:, cb, j * NQ:(j + 1) * NQ],
                        start=(cb == 0), stop=(cb == CB - 1))
                ob = obp.tile([P, NQ], F16, tag="ob", name=f"ob_{i}_{j}")
                if copy_eng == "s":
                    nc.scalar.copy(ob[:], ps[:])
                else:
                    nc.vector.tensor_copy(ob[:], ps[:])
                nc.sync.dma_start(
                    out[i * P:(i + 1) * P, j * NQ:(j + 1) * NQ], ob[:])

            def kb_list(qc):
                # diagonal blocks LAST: the pool queue is in-order, and a
                # unit's mask ops must come before its normalize broadcasts --
                # diag-first would make unit N+1's first PV wait (via its pool
                # mask, queued behind unit N's broadcast) for unit N's entire
                # normalize chain, serializing the units.
                return list(range(min(NKB, (qc + 1) * (NQ // P)))) if causal \
                    else list(range(NKB))

            # ---- one attention unit: scores+exp+PV+normalize for (hp, qc) ----
            # fill: list of closures emitting PE filler work, spread across the
            # kb loop so the tensor engine stays busy while exp runs.
            def unit(hp, qc, fill=()):
                kbs = kb_list(qc)
                q0 = qc * NQ
                fill = list(fill)
                # pop filler roughly evenly across the kb loop
                fill_at = {}
                for fi in range(len(fill)):
                    pos = min(len(kbs) - 1, (fi * len(kbs)) // max(1, len(fill)))
                    fill_at.setdefault(pos, []).append(fill[fi])

                otps = [ot_ps_pool.tile([HD + 1, NQ], F32, tag="ot",
                                        name=f"ot_{hp}_{qc}_{i}")
                        for i in range(2)]

                def finish(kb, stp2, qsl, diag, first, last):
                    pt = ptp.tile([P, 2, NQ], F16, tag="pt",
                                  name=f"pt_{hp}_{qc}_{kb}")
                    sflat = stp2.rearrange("p a b -> p (a b)")
                    pflat = pt.rearrange("p a b -> p (a b)")
                    # one exp covers both halves; the uncomputed middle
                    # columns of diagonal blocks are never read downstream
                    nc.scalar.activation(pflat[:, qsl:2 * NQ],
                                         sflat[:, qsl:2 * NQ],
                                         AF.Exp, scale=float(HD) ** -0.5)
                    for half in range(2):
                        h = hp * 2 + half
                        nc.tensor.matmul(
                            otps[half][:, qsl:NQ],
                            vaug[:, kb, h, :],
                            pt[:, half, qsl:NQ],
                            start=first, stop=last)

                pend = None
                for ki, kb in enumerate(kbs):
                    qsl = max(0, kb * P - q0) if causal else 0
                    diag = causal and kb * P >= q0
                    stp2 = st_ps_pool.tile([P, 2, NQ], F32, tag="st",
                                           name=f"st_{hp}_{qc}_{kb}")
                    for half in range(2):
                        hb = half * HD
                        nc.tensor.matmul(
                            stp2[:, half, qsl:NQ],
                            KT_sb[hb:hb + HD, hp, kb * P:(kb + 1) * P],
                            QT_sb[hb:hb + HD, hp, q0 + qsl:q0 + NQ],
                            start=True, stop=not diag)
                    if diag:
                        # causal mask: add -30000 strictly below the diagonal
                        # (both head planes in ONE matmul) so exp underflows
                        # those elements to zero
                        nc.tensor.matmul(
                            stp2[:, :, qsl:qsl + P],
                            ident_sb[:],
                            triB2_sb[:],
                            start=False, stop=True)
                    for f in fill_at.get(ki, ()):
                        f()
                    if pend is not None:
                        finish(*pend)
                    pend = (kb, stp2, qsl, diag, ki == 0, ki == len(kbs) - 1)
                finish(*pend)

                # normalize: one copy drains the PV accumulator to SBUF
                # (frees the PSUM bank early for the next unit's PV); then
                # 1/sums = exp(-ln(sums)) on the scalar engine (single table
                # set, see above), gpsimd partition-broadcast, one DVE multiply.
                last = (hp == 1 and qc == NT - 1)
                for half in range(2):
                    if not last:
                        osb = osp.tile([HD + 1, NQ], F16, tag="osb",
                                       name=f"os_{hp}_{qc}_{half}")
                        nc.vector.tensor_copy(osb[:], otps[half][:, :])
                        src_s, src_o = osb[HD:HD + 1, :], osb[0:HD, :]
                    else:
                        # final unit: skip the SBUF staging hop -- nothing
                        # needs the PSUM bank after this, and the tail is
                        # latency-critical
                        src_s, src_o = otps[half][HD:HD + 1, :], \
                            otps[half][0:HD, :]
                    lnr = rcp.tile([1, NQ], F32, tag="lnr",
                                   name=f"ln_{hp}_{qc}_{half}")
                    nc.scalar.activation(lnr[:], src_s, AF.Ln)
                    rcd = rcp.tile([1, NQ], F16, tag="rcd",
                                   name=f"rc_{hp}_{qc}_{half}")
                    nc.scalar.activation(rcd[:], lnr[:], AF.Exp, scale=-1.0)
                    bc = bcp.tile([HD, NQ], F16, tag="bc",
                                  name=f"bc_{hp}_{qc}_{half}")
                    nc.gpsimd.partition_broadcast(bc[:], rcd[:])
                    nc.vector.tensor_tensor(
                        attnT_sb[half * HD:(half + 1) * HD, hp,
                                 q0:q0 + NQ],
                        src_o, bc[:], MUL)

            # ---- emission: streaming pipeline ----
            # stage deps: unit(0,m) needs Q0/K0 chunks <= m and V blocks < 4(m+1);
            # unit(1,m) needs Q1/K1 chunks <= m; wo(qc) needs both hps normalized.
            def pq(cb, m, dma_swap=True):
                return lambda: proj_chunk(wqT_sb, QT_sb, cb, m, "q", dma_swap)

            def pk(cb, m, dma_swap=True):
                return lambda: proj_chunk(wkT_sb, KT_sb, cb, m, "k", dma_swap)

            def pv(iis):
                return lambda: proj_v(iis)

            def wo(i, j):
                return lambda: wo_unit(i, j)

            def xload(m):
                return lambda: nc.sync.dma_start(xT_sb[:, m], xT[:, m])

            def woload():
                return lambda: nc.sync.dma_start(woT_sb[:], woT[:])

            # projections are emitted one unit ahead of the unit that consumes
            # them, so their RoPE chain (DVE + swap DMAs) completes off the
            # critical path.  x3/wo load as sync-queue fill: pool-queue DMA
            # issues can't be ordered (software DGE fires immediately and
            # floods the front), and the scalar queue must stay clear for exp.
            pq(0, 0, False)(); pk(0, 0, False)(); pq(0, 1)(); pk(0, 1)()
            unit(0, 0, [pv([0, 1]), pv([2, 3]), pv([4, 5]),
                        pv([6, 7]), pq(0, 2)])
            unit(0, 1, [pv([8, 9]), xload(3), pv([10, 11]), pk(0, 2),
                        pq(0, 3)])
            unit(0, 2, [pv([12, 13]), pv([14, 15]), pk(0, 3), pq(1, 0)])
            unit(0, 3, [woload(), pk(1, 0), pq(1, 1), pk(1, 1)])
            unit(1, 0, [pq(1, 2), pk(1, 2)])
            # wo fills lag their normalize by ~a unit so they never head-of-
            # line block the PE queue behind an in-flight normalize chain
            unit(1, 1, [pq(1, 3), pk(1, 3), wo(0, 0)])
            unit(1, 2, [wo(0, 1), wo(1, 0), wo(1, 1), wo(2, 0), wo(2, 1),
                        wo(3, 0), wo(3, 1)])
            unit(1, 3, [wo(4, 0), wo(4, 1), wo(5, 0), wo(5, 1),
                        wo(6, 0), wo(6, 1), wo(7, 0), wo(7, 1),
                        wo(8, 0), wo(8, 1), wo(9, 0), wo(9, 1)])
            # i=10,11 only need norm(*,2): they give the PE work to chew
            # while the final unit's normalize chain runs
            for i in (10, 11):
                wo_unit(i, 0)
                wo_unit(i, 1)
            # tail: scalar engine is free after the last exp -- put the
            # drains there so the DVE isn't the tail pacer
            for i in range(12, 16):
                wo_unit(i, 0, copy_eng="s")
                wo_unit(i, 1, copy_eng="v")

    nc.compile()
    return nc


def _get_program(causal: bool):
    key = ("causal" if causal else "full")
    if key not in _prog_cache:
        _prog_cache[key] = _build_program(causal)
    return _prog_cache[key]


def _mask_kind(mask):
    m = np.asarray(mask)
    if m.ndim == 4:
        m = m[0, 0]
    if (m != 0).all():
        return False  # full attention
    trilm = np.tril(np.ones((m.shape[0], m.shape[1]), dtype=m.dtype))
    if np.array_equal(m, trilm):
        return True
    raise NotImplementedError("mask is neither all-ones nor causal tril")


def _make_in_maps(x, cos, sin, wq, wk, wv, wo):
    x = np.asarray(x, dtype=np.float32)
    cos = np.asarray(cos, dtype=np.float32)
    sin = np.asarray(sin, dtype=np.float32)
    wq = np.asarray(wq, dtype=np.float32)
    wk = np.asarray(wk, dtype=np.float32)
    wv = np.asarray(wv, dtype=np.float32)
    wo = np.asarray(wo, dtype=np.float32)

    # RoPE tables in transposed head-pair layout [128ch, T].
    # cos2T[c, t] = cos[t, c % 64]; sinS flips sign on the low half of each
    # head (the kernel swaps q's partner rows with a DMA, so the sin table is
    # in natural row order).
    cos2T = np.ascontiguousarray(cos[:T, :].T.astype(np.float16))   # [64, T]
    sgn = np.where(np.arange(HD) < (HD // 2), -1.0, 1.0).astype(np.float32)
    sinS = np.ascontiguousarray(
        (sin[:T, :].T * sgn[:, None]).astype(np.float16))      # [64, T]
    identm = np.eye(P, dtype=np.float16)
    triB = (np.tril(np.ones((P, P), np.float32), -1) * -30000.0).astype(np.float16)
    triB2 = np.ascontiguousarray(
        np.broadcast_to(triB[:, None, :], (P, 2, P)).copy())

    def pack_w(wT):
        # [D, CH] -> [128, DB, CH] with row p = wT[o*128+p, :]
        return np.ascontiguousarray(
            wT.reshape(D // P, P, -1).transpose(1, 0, 2).astype(np.float16))

    in_maps = []
    for core in range(NCORES):
        b = core // GROUPS
        g = core % GROUPS
        c0 = g * CH
        xb = x[b].T.astype(np.float16)                 # [D, T]
        xpack = np.ascontiguousarray(
            xb.reshape(D // P, P, T // NQ, NQ).transpose(1, 2, 0, 3))
        in_maps.append({
            "xT": xpack,                                        # [128, NT, DB, NQ]
            "wqT": pack_w(wq[c0:c0 + CH, :].T),
            "wkT": pack_w(wk[c0:c0 + CH, :].T),
            "wvT": pack_w(wv[c0:c0 + CH, :].T),
            "woT": np.ascontiguousarray(                       # [128, CB, D]
                wo[:, c0:c0 + CH].T.reshape(CH // P, P, D)
                .transpose(1, 0, 2).astype(np.float16)),
            "cosT": cos2T,
            "sinS": sinS,
            "ident": identm,
            "triB2": triB2,
        })
    return in_maps


def _run(inputs, trace=False):
    from concourse import bass_utils
    causal = _mask_kind(inputs["mask"])
    nc = _get_program(causal)
    in_maps = _make_in_maps(
        inputs["x"], inputs["cos"], inputs["sin"],
        inputs["wq"], inputs["wk"], inputs["wv"], inputs["wo"])
    if trace:
        _install_ntff_shim()
    res = bass_utils.run_bass_kernel_spmd(
        nc, in_maps, core_ids=list(range(NCORES)), trace=trace)
    outs = [r["out"] for r in res.results]
    full = np.empty((B, T, D), dtype=np.float32)
    for b in range(B):
        full[b] = outs[b * GROUPS].astype(np.float32)
        for g in range(1, GROUPS):
            full[b] += outs[b * GROUPS + g].astype(np.float32)
    return full, res


def kernel(**inputs):
    full, _ = _run(inputs, trace=False)
    return full


def kernel_profiled(**inputs):
    """Like kernel() but with NTFF tracing; returns (out, BassKernelResults)."""
    return _run(inputs, trace=True)
